# revision 47
# baseline (speedup 1.0000x reference)
"""Tensor-parallel LlamaAttention (GQA + RoPE + causal) for 8 trn2 NeuronCores.

Sharding: column-parallel q/k/v by head (NH/8 q-heads, NKV/8 kv-heads per
core), attention computed locally per head, per-512-token-chunk AllGather of
the (transposed) attention output overlapped with compute, then
column-parallel o_proj (each core computes a 512-wide output-column slice);
host concatenates slices.

Layout:
  xT[hb, tb] host-transposed into contiguous [128, 512] tiles (no PE
              transposes, max-efficiency DMA)
  qT/kT[d,t] from projection matmuls (lhsT=W block, rhs=xT block) + rope
  v[t, d]    natural layout (lhsT=xT block slice, rhs=Wv block)
  S^T[k, q]  = matmul(lhsT=kT slice, rhs=qT slice), diagonal tiles trimmed
               to the causally-live columns
  P^T        = exp(scale*S^T) on ACT; triangle mask on diag cols via DVE
  L          = ones-matmul PSUM accumulation; off-diagonal P tiles pair-
               summed on DVE (bf16) first so PE streams half the columns
  O^T[d, q] += matmul(lhsT=v tile, rhs=P^T)
  attnT      = O^T * reciprocal_approx_fast(L)

Schedule (the part that matters):
  tb0 runs hb-major from a deep PSUM pool so PE consumes tiles as the DMA
  warmup delivers them; batch-0 attention chunks (heads sequential, small
  PSUM footprint) interleave into batch-1's projection token-blocks so the
  serialized AllGathers start ~200us early; batch-1 chunks then interleave
  with o_proj of already-gathered chunks. A dummy warmup AllGather absorbs
  first-collective init. All collective outputs are Shared-addr-space DRAM.
"""

import math
import sys

import numpy as np

sys.path.insert(0, "/opt/trn_rl_repo")

import ml_dtypes  # noqa: E402

from concourse import bacc, mybir, tile  # noqa: E402
from concourse.bass_utils import run_bass_kernel_spmd  # noqa: E402

F32 = mybir.dt.float32
BF16 = mybir.dt.bfloat16
NCORES = 8
P = 128  # partitions / head dim
QB = 512  # q-block (PSUM free dim)
KB = 128  # k-block (contraction tile)
_CACHE = {}


def build_program(B, S, H, NH, NKV):
    """Build the per-core Bass program. All cores run the same program on
    different weight slices."""
    nc = bacc.Bacc("TRN2", num_devices=NCORES)

    BT = B * S  # total tokens
    NHC = NH // NCORES  # q heads per core
    NKC = NKV // NCORES  # kv heads per core
    assert NKC == 1
    DQ = NHC * P  # per-core q width
    HB = H // P  # h blocks
    TB = BT // QB  # token super-blocks
    QBB = S // QB  # q blocks per batch
    RB = QB // KB  # diag tiles per q block

    xT = nc.declare_dram_parameter("xT", [HB, BT // QB, P, QB], BF16, isOutput=False)
    wq_c = nc.declare_dram_parameter("wq_c", [H, DQ], BF16, isOutput=False)
    wk_c = nc.declare_dram_parameter("wk_c", [H, P], BF16, isOutput=False)
    wv_c = nc.declare_dram_parameter("wv_c", [H, P], BF16, isOutput=False)
    wo_c = nc.declare_dram_parameter("wo_c", [H, DQ], BF16, isOutput=False)
    cos_t = nc.declare_dram_parameter("cos_t", [BT // QB, P, QB], F32, isOutput=False)
    sinx_t = nc.declare_dram_parameter("sinx_t", [BT // QB, P, QB], F32, isOutput=False)
    masks_t = nc.declare_dram_parameter("masks_t", [RB, P, QB], BF16, isOutput=False)
    ones_t = nc.declare_dram_parameter("ones_t", [P, P], BF16, isOutput=False)
    y_c = nc.declare_dram_parameter("y_c", [BT, DQ], F32, isOutput=True)

    scale = 1.0 / math.sqrt(P)
    nchunks = B * QBB  # attention/o_proj pipeline chunks (QB tokens each)

    with tile.TileContext(nc) as tc:
        with (
            tc.tile_pool(name="dram", bufs=1, space="DRAM") as dram,
            tc.tile_pool(name="const", bufs=1) as constp,
            tc.tile_pool(name="persist", bufs=1) as persist,
            tc.tile_pool(name="pP", bufs=6) as p_p,
            tc.tile_pool(name="pPair", bufs=3) as pp_p,
            tc.tile_pool(name="rinvp", bufs=2) as r_p,
            tc.tile_pool(name="aout", bufs=4) as ao_p,
        ):
            # dummy collective during phase 1 absorbs first-collective init
            cc_warm_in = dram.tile([P, 8], BF16, tag="ccwi", name="ccwi")
            cc_warm_out = dram.tile(
                [NCORES * P, 8], BF16, tag="ccwo", name="ccwo", addr_space="Shared"
            )
            attn_loc = [
                dram.tile([DQ, QB], BF16, tag=f"attn_loc{c}", name=f"attn_loc{c}")
                for c in range(nchunks)
            ]
            attn_full = [
                dram.tile(
                    [NCORES * DQ, QB],
                    BF16,
                    tag=f"attn_full{c}",
                    name=f"attn_full{c}",
                    addr_space="Shared",
                )
                for c in range(nchunks)
            ]

            # persistent per-core activations (bf16)
            qT = [persist.tile([P, BT], BF16, tag=f"qT{i}", name=f"qT{i}") for i in range(NHC)]
            kT = persist.tile([P, BT], BF16, tag="kT")
            vt = [persist.tile([P, P], BF16, tag=f"v{i}", name=f"v{i}") for i in range(BT // P)]

            HG2 = min(2, NHC)

            def p2_chunk(c, pss_p, pso_p, psl_p, HG, ones_bf, mask_sb):
                b, qb = divmod(c, QBB)
                q0 = b * S + qb * QB
                nkb = (qb + 1) * RB
                for hp in range(NHC // HG):
                    heads = range(hp * HG, (hp + 1) * HG)
                    o_ps = {
                        h: pso_p.tile([P, QB], F32, tag=f"ops{h % HG}", name=f"ops{h % HG}")
                        for h in heads
                    }
                    l_ps = {
                        h: psl_p.tile([P, QB], F32, tag=f"lps{h % HG}", name=f"lps{h % HG}")
                        for h in heads
                    }
                    # L bookkeeping: off-diagonal P tiles are summed in pairs
                    # on DVE (bf16) so PE streams half the columns through the
                    # ones-matmul; diagonal tiles go straight to PE (trimmed).
                    lfirst = {h: True for h in heads}
                    pend = {h: None for h in heads}

                    def l_mm(h, rhs, w0, stop):
                        nc.tensor.matmul(
                            l_ps[h][:, w0:QB],
                            ones_bf,
                            rhs[:, w0:QB],
                            start=lfirst[h],
                            stop=stop,
                        )
                        lfirst[h] = False

                    for kb in range(nkb):
                        k0 = b * S + kb * KB
                        o = kb - qb * RB  # >=0 -> diagonal tile
                        # causal: first o*KB columns of this q-block see
                        # nothing from this k-tile; compute only the rest.
                        w0 = o * KB if o > 0 else 0
                        p_sbs = {}
                        for h in heads:
                            s_ps = pss_p.tile([P, QB], F32, tag="sps")
                            nc.tensor.matmul(
                                s_ps[:, w0:QB],
                                kT[:, k0 : k0 + KB],
                                qT[h][:, q0 + w0 : q0 + QB],
                                start=True,
                                stop=True,
                            )
                            p_sb = p_p.tile([P, QB], BF16, tag="P")
                            nc.scalar.activation(
                                p_sb[:, w0:QB],
                                s_ps[:, w0:QB],
                                mybir.ActivationFunctionType.Exp,
                                scale=scale,
                            )
                            if o >= 0:
                                # triangle mask on the 128 diagonal cols
                                nc.vector.tensor_tensor(
                                    p_sb[:, w0 : w0 + KB],
                                    p_sb[:, w0 : w0 + KB],
                                    mask_sb,
                                    mybir.AluOpType.mult,
                                )
                            p_sbs[h] = p_sb
                        for h in heads:
                            nc.tensor.matmul(
                                o_ps[h][:, w0:QB],
                                vt[(b * S + kb * KB) // P],
                                p_sbs[h][:, w0:QB],
                                start=(kb == 0),
                                stop=(kb == nkb - 1),
                            )
                            if o >= 0:
                                if pend[h] is not None:  # odd off-diag count
                                    l_mm(h, pend[h], 0, False)
                                    pend[h] = None
                                l_mm(h, p_sbs[h], w0, kb == nkb - 1)
                            elif pend[h] is None:
                                pend[h] = p_sbs[h]
                            else:
                                ppair = pp_p.tile([P, QB], BF16, tag="ppair")
                                nc.vector.tensor_tensor(
                                    ppair, pend[h], p_sbs[h], mybir.AluOpType.add
                                )
                                pend[h] = None
                                l_mm(h, ppair, 0, False)
                    for h in heads:
                        rinv = r_p.tile([P, QB], F32, tag="rinv")
                        nc.vector.reciprocal_approx_fast(rinv, l_ps[h])
                        attn_sb = ao_p.tile([P, QB], BF16, tag="attn")
                        nc.vector.tensor_tensor(
                            attn_sb, o_ps[h], rinv, mybir.AluOpType.mult
                        )
                        nc.sync.dma_start(
                            out=attn_loc[c][h * P : (h + 1) * P, :], in_=attn_sb
                        )
                nc.gpsimd.collective_compute(
                    "AllGather",
                    mybir.AluOpType.bypass,
                    replica_groups=[list(range(NCORES))],
                    ins=[attn_loc[c][:, :]],
                    outs=[attn_full[c][:, :]],
                )

            # ---------------- phase 1: q/k/v projections + rope, with batch-0
            # attention chunks (and their collectives) interleaved into the
            # second half so the AllGathers start as early as possible.
            with (
                tc.tile_pool(name="xs", bufs=2 * HB + 4) as xs_p,
                tc.tile_pool(name="wqkv", bufs=1) as w_p,
                tc.tile_pool(name="tabs", bufs=2) as tab_p,
                tc.tile_pool(name="ropetmp", bufs=4) as rt_p,
                tc.tile_pool(name="psq", bufs=2, space="PSUM") as psq_p,
                tc.tile_pool(name="psk", bufs=1, space="PSUM") as psk_p,
                tc.tile_pool(name="psv", bufs=1, space="PSUM") as psv_p,
            ):
                # DMA issue order matters for warmup: first token-block's x
                # tiles and wq interleaved so PE can start right away, then
                # the second token-block's x, then the remaining weights.
                cos_sb0 = tab_p.tile([P, QB], F32, tag="cos", name="cos0")
                sinx_sb0 = tab_p.tile([P, QB], F32, tag="sinx", name="sinx0")
                nc.sync.dma_start(out=cos_sb0, in_=cos_t[0])
                nc.sync.dma_start(out=sinx_sb0, in_=sinx_t[0])
                wq_sb = [w_p.tile([P, DQ], BF16, tag=f"wq{i}", name=f"wq{i}") for i in range(HB)]
                wk_sb = [w_p.tile([P, P], BF16, tag=f"wk{i}", name=f"wk{i}") for i in range(HB)]
                wv_sb = [w_p.tile([P, P], BF16, tag=f"wv{i}", name=f"wv{i}") for i in range(HB)]

                def load_x(tb):
                    cos_sb = tab_p.tile([P, QB], F32, tag="cos", name=f"cos{tb}")
                    sinx_sb = tab_p.tile([P, QB], F32, tag="sinx", name=f"sinx{tb}")
                    nc.sync.dma_start(out=cos_sb, in_=cos_t[tb])
                    nc.sync.dma_start(out=sinx_sb, in_=sinx_t[tb])
                    xts = []
                    for hb in range(HB):
                        xi = xs_p.tile([P, QB], BF16, tag="xs", name=f"x{tb}_{hb}")
                        nc.sync.dma_start(out=xi, in_=xT[hb, tb])
                        xts.append(xi)
                    return cos_sb, sinx_sb, xts

                xbuf = {}
                xts0 = []
                for hb in range(HB):
                    xi = xs_p.tile([P, QB], BF16, tag="xs", name=f"x0_{hb}")
                    # split issue across both DGE sequencers for warmup rate
                    nc.sync.dma_start(out=xi, in_=xT[hb, 0])
                    xts0.append(xi)
                    nc.scalar.dma_start(out=wq_sb[hb], in_=wq_c[hb * P : (hb + 1) * P, :])
                    nc.scalar.dma_start(out=wk_sb[hb], in_=wk_c[hb * P : (hb + 1) * P, :])
                    nc.scalar.dma_start(out=wv_sb[hb], in_=wv_c[hb * P : (hb + 1) * P, :])
                xbuf[0] = (cos_sb0, sinx_sb0, xts0)
                if TB > 1:
                    xbuf[1] = load_x(1)
                # constants for phase 2 (small; needed later)
                ones_bf = constp.tile([P, P], BF16, tag="ones")
                nc.sync.dma_start(out=ones_bf, in_=ones_t[:, :])
                mask_sb = constp.tile([P, P], BF16, tag="mask")
                nc.sync.dma_start(out=mask_sb, in_=masks_t[0, :, 0:P])
                nc.sync.dma_start(out=cc_warm_in[:, :], in_=ones_t[:, 0:8])
                nc.gpsimd.collective_compute(
                    "AllGather",
                    mybir.AluOpType.bypass,
                    replica_groups=[list(range(NCORES))],
                    ins=[cc_warm_in[:, :]],
                    outs=[cc_warm_out[:, :]],
                )

                def rope(dst, ps, cos_sb, sinx_sb):
                    """dst[:, t0:t0+QB] = ps*cos + shift64(ps)*sinx (all [128,QB])"""
                    t1 = rt_p.tile([P, QB], F32, tag="ropet1")
                    t2 = rt_p.tile([P, QB], F32, tag="ropet2")
                    nc.vector.tensor_tensor(t1, ps, cos_sb, mybir.AluOpType.mult)
                    h = P // 2
                    nc.vector.tensor_tensor(
                        t2[0:h], ps[h:P], sinx_sb[0:h], mybir.AluOpType.mult
                    )
                    nc.vector.tensor_tensor(
                        t2[h:P], ps[0:h], sinx_sb[h:P], mybir.AluOpType.mult
                    )
                    nc.vector.tensor_tensor(dst, t1, t2, mybir.AluOpType.add)

                def p1_tb(tb, qpool, hbmajor=False):
                    t0 = tb * QB
                    if tb + 1 < TB and tb + 1 not in xbuf:
                        xbuf[tb + 1] = load_x(tb + 1)
                    cos_sb, sinx_sb, xts = xbuf.pop(tb)

                    if hbmajor:
                        # hb-major so PE consumes x/w tiles as DMA delivers
                        # them (warmup); needs all NHC q accumulators live.
                        q_pss = [
                            qpool.tile([P, QB], F32, tag="qps", name=f"qps{dq}")
                            for dq in range(NHC)
                        ]
                        k_ps = psk_p.tile([P, QB], F32, tag="kps")
                        for hb in range(HB):
                            for dq in range(NHC):
                                nc.tensor.matmul(
                                    q_pss[dq],
                                    wq_sb[hb][:, dq * P : (dq + 1) * P],
                                    xts[hb],
                                    start=(hb == 0),
                                    stop=(hb == HB - 1),
                                )
                            nc.tensor.matmul(
                                k_ps,
                                wk_sb[hb],
                                xts[hb],
                                start=(hb == 0),
                                stop=(hb == HB - 1),
                            )
                        for dq in range(NHC):
                            rope(qT[dq][:, t0 : t0 + QB], q_pss[dq], cos_sb, sinx_sb)
                        rope(kT[:, t0 : t0 + QB], k_ps, cos_sb, sinx_sb)
                    else:
                        # q projections (per 128-wide d block) + rope
                        for dq in range(NHC):
                            q_ps = qpool.tile([P, QB], F32, tag="qps")
                            for hb in range(HB):
                                nc.tensor.matmul(
                                    q_ps,
                                    wq_sb[hb][:, dq * P : (dq + 1) * P],
                                    xts[hb],
                                    start=(hb == 0),
                                    stop=(hb == HB - 1),
                                )
                            rope(qT[dq][:, t0 : t0 + QB], q_ps, cos_sb, sinx_sb)
                        # k projection + rope
                        k_ps = psk_p.tile([P, QB], F32, tag="kps")
                        for hb in range(HB):
                            nc.tensor.matmul(
                                k_ps,
                                wk_sb[hb],
                                xts[hb],
                                start=(hb == 0),
                                stop=(hb == HB - 1),
                            )
                        rope(kT[:, t0 : t0 + QB], k_ps, cos_sb, sinx_sb)
                    # v projection (natural [t, d] layout)
                    for i in range(QB // P):
                        v_ps = psv_p.tile([P, P], F32, tag="vps")
                        for hb in range(HB):
                            nc.tensor.matmul(
                                v_ps,
                                xts[hb][:, i * P : (i + 1) * P],
                                wv_sb[hb],
                                start=(hb == 0),
                                stop=(hb == HB - 1),
                            )
                        nc.scalar.copy(vt[tb * (QB // P) + i], v_ps)

                # batch-0 token blocks first (deep psum pool for warmup),
                # then batch-1 blocks interleaved with batch-0 attention
                # chunks (big q-blocks first)
                with tc.tile_pool(name="psqw", bufs=NHC, space="PSUM") as psqw_p:
                    for tb in range(QBB):
                        p1_tb(tb, psqw_p, hbmajor=(tb == 0))
                with (
                    tc.tile_pool(name="psSa", bufs=2, space="PSUM") as pssa_p,
                    tc.tile_pool(name="psOa", bufs=1, space="PSUM") as psoa_p,
                    tc.tile_pool(name="psLa", bufs=1, space="PSUM") as psla_p,
                ):
                    b0_chunks = list(range(QBB - 1, -1, -1))
                    rest_tbs = list(range(QBB, TB))
                    for j in range(max(len(b0_chunks), len(rest_tbs))):
                        if j < len(b0_chunks):
                            p2_chunk(b0_chunks[j], pssa_p, psoa_p, psla_p, 1, ones_bf, mask_sb)
                        if j < len(rest_tbs):
                            p1_tb(rest_tbs[j], psq_p)

            # ---------------- phase 2b+3: batch-1 attention chunks (paired
            # heads) interleaved with o_proj for the already-gathered chunks.
            with (
                tc.tile_pool(name="wo", bufs=1) as wo_p,
                tc.tile_pool(name="astr", bufs=HB + 4) as as_p,
                tc.tile_pool(name="yout", bufs=4) as y_p,
                tc.tile_pool(name="psS", bufs=2, space="PSUM") as pss_p,
                tc.tile_pool(name="psO", bufs=1, space="PSUM") as pso_p,
                tc.tile_pool(name="psL", bufs=1, space="PSUM") as psl_p,
                tc.tile_pool(name="psY", bufs=2, space="PSUM") as psy_p,
            ):
                wo_sb = [wo_p.tile([P, DQ], BF16, tag=f"wo{i}", name=f"wo{i}") for i in range(HB)]
                for hb in range(HB):
                    nc.sync.dma_start(out=wo_sb[hb], in_=wo_c[hb * P : (hb + 1) * P, :])

                def p3_chunk(c):
                    t0 = c * QB
                    at = []
                    for ha in range(HB):
                        a = as_p.tile([P, QB], BF16, tag="astream")
                        nc.scalar.dma_start(
                            out=a, in_=attn_full[c][ha * P : (ha + 1) * P, :]
                        )
                        at.append(a)
                    for i in range(QB // P):
                        y_ps = psy_p.tile([P, DQ], F32, tag="yps")
                        for ha in range(HB):
                            nc.tensor.matmul(
                                y_ps,
                                at[ha][:, i * P : (i + 1) * P],
                                wo_sb[ha],
                                start=(ha == 0),
                                stop=(ha == HB - 1),
                            )
                        y_sb = y_p.tile([P, DQ], F32, tag="ysb")
                        if i % 2 == 0:
                            nc.scalar.copy(y_sb, y_ps)
                        else:
                            nc.vector.tensor_copy(y_sb, y_ps)
                        nc.scalar.dma_start(
                            out=y_c[t0 + i * P : t0 + (i + 1) * P, :], in_=y_sb
                        )

                b0_done = list(range(QBB - 1, -1, -1))
                later = [
                    b * QBB + qb
                    for b in range(1, B)
                    for qb in range(QBB - 1, -1, -1)
                ]
                p3_queue = b0_done + later
                for j, c in enumerate(later):
                    p2_chunk(c, pss_p, pso_p, psl_p, HG2, ones_bf, mask_sb)
                    if j < len(b0_done):
                        p3_chunk(b0_done[j])
                for c in p3_queue[len(later):]:
                    p3_chunk(c)
    nc.finalize()
    return nc


def _prep_inputs(hidden_states, wq, wk, wv, wo, position_ids, B, S, H, NH, NKV):
    """Host-side: bf16 casts, x transpose, rope tables, causal masks,
    per-core slices."""
    BT = B * S
    NHC = NH // NCORES
    DQ = NHC * P
    RB = QB // KB

    bf = ml_dtypes.bfloat16
    HB = H // P
    TBc = BT // QB
    x2 = np.asarray(hidden_states).reshape(BT, H).astype(bf)
    # blocked transpose: xT[hb, tb] is a contiguous [128, QB] tile
    xT = np.ascontiguousarray(
        x2.T.reshape(HB, P, TBc, QB).transpose(0, 2, 1, 3)
    )
    wq_b, wk_b, wv_b, wo_b = (np.asarray(w).astype(bf) for w in (wq, wk, wv, wo))

    # rope tables in transposed layout, blocked per token superblock
    half = P // 2
    inv_freq = 1.0 / (10000.0 ** (np.arange(half, dtype=np.float64) / half))
    pos = np.asarray(position_ids).astype(np.float64).reshape(BT)  # [b*S+s]
    ang = pos[None, :] * inv_freq[:, None]  # [64, BT]
    cos_t = np.concatenate([np.cos(ang), np.cos(ang)], 0).astype(np.float32)
    sinx_t = np.concatenate([-np.sin(ang), np.sin(ang)], 0).astype(np.float32)
    cos_t = np.ascontiguousarray(cos_t.reshape(P, TBc, QB).transpose(1, 0, 2))
    sinx_t = np.ascontiguousarray(sinx_t.reshape(P, TBc, QB).transpose(1, 0, 2))

    # diagonal-block causal masks: mask[o][k, q] = 1 if o*KB + k <= q
    kk = np.arange(KB)[None, :, None]
    qq = np.arange(QB)[None, None, :]
    oo = np.arange(RB)[:, None, None]
    masks_t = ((oo * KB + kk) <= qq).astype(bf)
    ones_t = np.ones((P, P), bf)

    in_maps = []
    for c in range(NCORES):
        in_maps.append(
            {
                "xT": xT,
                "wq_c": np.ascontiguousarray(wq_b[:, c * DQ : (c + 1) * DQ]),
                "wk_c": np.ascontiguousarray(wk_b[:, c * P : (c + 1) * P]),
                "wv_c": np.ascontiguousarray(wv_b[:, c * P : (c + 1) * P]),
                "wo_c": np.ascontiguousarray(wo_b[:, c * DQ : (c + 1) * DQ]),
                "cos_t": cos_t,
                "sinx_t": sinx_t,
                "masks_t": masks_t,
                "ones_t": ones_t,
            }
        )
    return in_maps


def run(hidden_states, wq, wk, wv, wo, position_ids, B, S, H, NH, NKV, trace=False):
    key = (B, S, H, NH, NKV)
    if key not in _CACHE:
        _CACHE[key] = build_program(B, S, H, NH, NKV)
    nc = _CACHE[key]
    in_maps = _prep_inputs(
        hidden_states, wq, wk, wv, wo, position_ids, B, S, H, NH, NKV
    )
    res = run_bass_kernel_spmd(nc, in_maps, core_ids=list(range(NCORES)), trace=trace)
    y = np.concatenate([res.results[c]["y_c"] for c in range(NCORES)], axis=1)
    out = y.reshape(B, S, NH * P).astype(np.float32)
    return (out, res) if trace else (out, None)


def kernel(hidden_states, wq, wk, wv, wo, position_ids):
    out, _ = run(
        hidden_states, wq, wk, wv, wo, position_ids, 2, 2048, 4096, 32, 8
    )
    return out


# revision 48
# speedup vs baseline: 1.0419x; 1.0419x over previous
"""Tensor-parallel LlamaAttention (GQA + RoPE + causal) for 8 trn2 NeuronCores.

Sharding: column-parallel q/k/v by head (NH/8 q-heads, NKV/8 kv-heads per
core), attention computed locally per head, per-512-token-chunk AllGather of
the (transposed) attention output overlapped with compute, then
column-parallel o_proj (each core computes a 512-wide output-column slice);
host concatenates slices.

Layout:
  xT[hb, tb] host-transposed into contiguous [128, 512] tiles (no PE
              transposes, max-efficiency DMA)
  qT/kT[d,t] from projection matmuls (lhsT=W block, rhs=xT block) + rope
  v[t, d]    natural layout (lhsT=xT block slice, rhs=Wv block)
  S^T[k, q]  = matmul(lhsT=kT slice, rhs=qT slice), diagonal tiles trimmed
               to the causally-live columns
  P^T        = exp(scale*S^T) on ACT; triangle mask on diag cols via DVE
  L          = ones-matmul PSUM accumulation; off-diagonal P tiles pair-
               summed on DVE (bf16) first so PE streams half the columns
  O^T[d, q] += matmul(lhsT=v tile, rhs=P^T)
  attnT      = O^T * reciprocal_approx_fast(L)

Schedule (the part that matters):
  tb0 runs hb-major from a deep PSUM pool so PE consumes tiles as the DMA
  warmup delivers them; batch-0 attention chunks (heads sequential, small
  PSUM footprint) interleave into batch-1's projection token-blocks so the
  serialized AllGathers start ~200us early; batch-1 chunks then interleave
  with o_proj of already-gathered chunks. A dummy warmup AllGather absorbs
  first-collective init. All collective outputs are Shared-addr-space DRAM.
"""

import math
import sys

import numpy as np

sys.path.insert(0, "/opt/trn_rl_repo")

import ml_dtypes  # noqa: E402

from concourse import bacc, mybir, tile  # noqa: E402
from concourse.bass_utils import run_bass_kernel_spmd  # noqa: E402

F32 = mybir.dt.float32
BF16 = mybir.dt.bfloat16
NCORES = 8
P = 128  # partitions / head dim
QB = 512  # q-block (PSUM free dim)
KB = 128  # k-block (contraction tile)
_CACHE = {}


def build_program(B, S, H, NH, NKV):
    """Build the per-core Bass program. All cores run the same program on
    different weight slices."""
    nc = bacc.Bacc("TRN2", num_devices=NCORES)

    BT = B * S  # total tokens
    NHC = NH // NCORES  # q heads per core
    NKC = NKV // NCORES  # kv heads per core
    assert NKC == 1
    DQ = NHC * P  # per-core q width
    HB = H // P  # h blocks
    TB = BT // QB  # token super-blocks
    QBB = S // QB  # q blocks per batch
    RB = QB // KB  # diag tiles per q block

    xT = nc.declare_dram_parameter("xT", [HB, BT // QB, P, QB], BF16, isOutput=False)
    wq_c = nc.declare_dram_parameter("wq_c", [H, DQ], BF16, isOutput=False)
    wk_c = nc.declare_dram_parameter("wk_c", [H, P], BF16, isOutput=False)
    wv_c = nc.declare_dram_parameter("wv_c", [H, P], BF16, isOutput=False)
    wo_c = nc.declare_dram_parameter("wo_c", [H, DQ], BF16, isOutput=False)
    cos_t = nc.declare_dram_parameter("cos_t", [BT // QB, P, QB], F32, isOutput=False)
    sinx_t = nc.declare_dram_parameter("sinx_t", [BT // QB, P, QB], F32, isOutput=False)
    masks_t = nc.declare_dram_parameter("masks_t", [RB, P, QB], BF16, isOutput=False)
    ones_t = nc.declare_dram_parameter("ones_t", [P, P], BF16, isOutput=False)
    y_c = nc.declare_dram_parameter("y_c", [BT, DQ], F32, isOutput=True)

    scale = 1.0 / math.sqrt(P)
    nchunks = B * QBB  # attention/o_proj pipeline chunks (QB tokens each)

    with tile.TileContext(nc) as tc:
        with (
            tc.tile_pool(name="dram", bufs=1, space="DRAM") as dram,
            tc.tile_pool(name="const", bufs=1) as constp,
            tc.tile_pool(name="persist", bufs=1) as persist,
            tc.tile_pool(name="pP", bufs=6) as p_p,
            tc.tile_pool(name="pPair", bufs=3) as pp_p,
            tc.tile_pool(name="rinvp", bufs=2) as r_p,
            tc.tile_pool(name="aout", bufs=4) as ao_p,
        ):
            # dummy collective during phase 1 absorbs first-collective init
            cc_warm_in = dram.tile([P, 8], BF16, tag="ccwi", name="ccwi")
            cc_warm_out = dram.tile(
                [NCORES * P, 8], BF16, tag="ccwo", name="ccwo", addr_space="Shared"
            )
            attn_loc = [
                dram.tile([DQ, QB], BF16, tag=f"attn_loc{c}", name=f"attn_loc{c}")
                for c in range(nchunks)
            ]
            attn_full = [
                dram.tile(
                    [NCORES * DQ, QB],
                    BF16,
                    tag=f"attn_full{c}",
                    name=f"attn_full{c}",
                    addr_space="Shared",
                )
                for c in range(nchunks)
            ]

            # persistent per-core activations (bf16)
            qT = [persist.tile([P, BT], BF16, tag=f"qT{i}", name=f"qT{i}") for i in range(NHC)]
            kT = persist.tile([P, BT], BF16, tag="kT")
            vt = [persist.tile([P, P], BF16, tag=f"v{i}", name=f"v{i}") for i in range(BT // P)]

            HG2 = min(2, NHC)

            def p2_chunk(c, pss_p, pso_p, psl_p, HG, ones_bf, mask_sb):
                b, qb = divmod(c, QBB)
                q0 = b * S + qb * QB
                nkb = (qb + 1) * RB
                for hp in range(NHC // HG):
                    heads = range(hp * HG, (hp + 1) * HG)
                    o_ps = {
                        h: pso_p.tile([P, QB], F32, tag=f"ops{h % HG}", name=f"ops{h % HG}")
                        for h in heads
                    }
                    l_ps = {
                        h: psl_p.tile([P, QB], F32, tag=f"lps{h % HG}", name=f"lps{h % HG}")
                        for h in heads
                    }
                    # L bookkeeping: off-diagonal P tiles are summed in pairs
                    # on DVE (bf16) so PE streams half the columns through the
                    # ones-matmul; diagonal tiles go straight to PE (trimmed).
                    lfirst = {h: True for h in heads}
                    pend = {h: None for h in heads}

                    def l_mm(h, rhs, w0, stop):
                        nc.tensor.matmul(
                            l_ps[h][:, w0:QB],
                            ones_bf,
                            rhs[:, w0:QB],
                            start=lfirst[h],
                            stop=stop,
                        )
                        lfirst[h] = False

                    for kb in range(nkb):
                        k0 = b * S + kb * KB
                        o = kb - qb * RB  # >=0 -> diagonal tile
                        # causal: first o*KB columns of this q-block see
                        # nothing from this k-tile; compute only the rest.
                        w0 = o * KB if o > 0 else 0
                        p_sbs = {}
                        for h in heads:
                            s_ps = pss_p.tile([P, QB], F32, tag="sps")
                            nc.tensor.matmul(
                                s_ps[:, w0:QB],
                                kT[:, k0 : k0 + KB],
                                qT[h][:, q0 + w0 : q0 + QB],
                                start=True,
                                stop=True,
                            )
                            p_sb = p_p.tile([P, QB], BF16, tag="P")
                            nc.scalar.activation(
                                p_sb[:, w0:QB],
                                s_ps[:, w0:QB],
                                mybir.ActivationFunctionType.Exp,
                                scale=scale,
                            )
                            if o >= 0:
                                # triangle mask on the 128 diagonal cols
                                nc.vector.tensor_tensor(
                                    p_sb[:, w0 : w0 + KB],
                                    p_sb[:, w0 : w0 + KB],
                                    mask_sb,
                                    mybir.AluOpType.mult,
                                )
                            p_sbs[h] = p_sb
                        for h in heads:
                            nc.tensor.matmul(
                                o_ps[h][:, w0:QB],
                                vt[(b * S + kb * KB) // P],
                                p_sbs[h][:, w0:QB],
                                start=(kb == 0),
                                stop=(kb == nkb - 1),
                            )
                            if o >= 0:
                                if pend[h] is not None:  # odd off-diag count
                                    l_mm(h, pend[h], 0, False)
                                    pend[h] = None
                                l_mm(h, p_sbs[h], w0, kb == nkb - 1)
                            elif pend[h] is None:
                                pend[h] = p_sbs[h]
                            else:
                                ppair = pp_p.tile([P, QB], BF16, tag="ppair")
                                nc.vector.tensor_tensor(
                                    ppair, pend[h], p_sbs[h], mybir.AluOpType.add
                                )
                                pend[h] = None
                                l_mm(h, ppair, 0, False)
                    for h in heads:
                        rinv = r_p.tile([P, QB], F32, tag="rinv")
                        nc.vector.reciprocal_approx_fast(rinv, l_ps[h])
                        attn_sb = ao_p.tile([P, QB], BF16, tag="attn")
                        nc.vector.tensor_tensor(
                            attn_sb, o_ps[h], rinv, mybir.AluOpType.mult
                        )
                        nc.sync.dma_start(
                            out=attn_loc[c][h * P : (h + 1) * P, :], in_=attn_sb
                        )
                nc.gpsimd.collective_compute(
                    "AllGather",
                    mybir.AluOpType.bypass,
                    replica_groups=[list(range(NCORES))],
                    ins=[attn_loc[c][:, :]],
                    outs=[attn_full[c][:, :]],
                )

            # ---------------- phase 1: q/k/v projections + rope, with batch-0
            # attention chunks (and their collectives) interleaved into the
            # second half so the AllGathers start as early as possible.
            with (
                tc.tile_pool(name="xs", bufs=2 * HB + 4) as xs_p,
                tc.tile_pool(name="wqkv", bufs=1) as w_p,
                tc.tile_pool(name="tabs", bufs=2) as tab_p,
                tc.tile_pool(name="ropetmp", bufs=4) as rt_p,
                tc.tile_pool(name="psq", bufs=2, space="PSUM") as psq_p,
                tc.tile_pool(name="psk", bufs=1, space="PSUM") as psk_p,
                tc.tile_pool(name="psv", bufs=1, space="PSUM") as psv_p,
            ):
                # DMA issue order matters for warmup: first token-block's x
                # tiles and wq interleaved so PE can start right away, then
                # the second token-block's x, then the remaining weights.
                cos_sb0 = tab_p.tile([P, QB], F32, tag="cos", name="cos0")
                sinx_sb0 = tab_p.tile([P, QB], F32, tag="sinx", name="sinx0")
                nc.sync.dma_start(out=cos_sb0, in_=cos_t[0])
                nc.sync.dma_start(out=sinx_sb0, in_=sinx_t[0])
                wq_sb = [w_p.tile([P, DQ], BF16, tag=f"wq{i}", name=f"wq{i}") for i in range(HB)]
                wk_sb = [w_p.tile([P, P], BF16, tag=f"wk{i}", name=f"wk{i}") for i in range(HB)]
                wv_sb = [w_p.tile([P, P], BF16, tag=f"wv{i}", name=f"wv{i}") for i in range(HB)]

                def load_x(tb):
                    cos_sb = tab_p.tile([P, QB], F32, tag="cos", name=f"cos{tb}")
                    sinx_sb = tab_p.tile([P, QB], F32, tag="sinx", name=f"sinx{tb}")
                    nc.sync.dma_start(out=cos_sb, in_=cos_t[tb])
                    nc.sync.dma_start(out=sinx_sb, in_=sinx_t[tb])
                    xts = []
                    for hb in range(HB):
                        xi = xs_p.tile([P, QB], BF16, tag="xs", name=f"x{tb}_{hb}")
                        nc.sync.dma_start(out=xi, in_=xT[hb, tb])
                        xts.append(xi)
                    return cos_sb, sinx_sb, xts

                xbuf = {}
                xts0 = []
                for hb in range(HB):
                    xi = xs_p.tile([P, QB], BF16, tag="xs", name=f"x0_{hb}")
                    nc.sync.dma_start(out=xi, in_=xT[hb, 0])
                    xts0.append(xi)
                    nc.sync.dma_start(out=wq_sb[hb], in_=wq_c[hb * P : (hb + 1) * P, :])
                    nc.sync.dma_start(out=wk_sb[hb], in_=wk_c[hb * P : (hb + 1) * P, :])
                xbuf[0] = (cos_sb0, sinx_sb0, xts0)
                if TB > 1:
                    xbuf[1] = load_x(1)
                for hb in range(HB):
                    nc.sync.dma_start(out=wv_sb[hb], in_=wv_c[hb * P : (hb + 1) * P, :])
                # constants for phase 2 (small; needed later)
                ones_bf = constp.tile([P, P], BF16, tag="ones")
                nc.sync.dma_start(out=ones_bf, in_=ones_t[:, :])
                mask_sb = constp.tile([P, P], BF16, tag="mask")
                nc.sync.dma_start(out=mask_sb, in_=masks_t[0, :, 0:P])
                nc.sync.dma_start(out=cc_warm_in[:, :], in_=ones_t[:, 0:8])
                nc.gpsimd.collective_compute(
                    "AllGather",
                    mybir.AluOpType.bypass,
                    replica_groups=[list(range(NCORES))],
                    ins=[cc_warm_in[:, :]],
                    outs=[cc_warm_out[:, :]],
                )

                def rope(dst, ps, cos_sb, sinx_sb):
                    """dst[:, t0:t0+QB] = ps*cos + shift64(ps)*sinx (all [128,QB])"""
                    t1 = rt_p.tile([P, QB], F32, tag="ropet1")
                    t2 = rt_p.tile([P, QB], F32, tag="ropet2")
                    nc.vector.tensor_tensor(t1, ps, cos_sb, mybir.AluOpType.mult)
                    h = P // 2
                    nc.vector.tensor_tensor(
                        t2[0:h], ps[h:P], sinx_sb[0:h], mybir.AluOpType.mult
                    )
                    nc.vector.tensor_tensor(
                        t2[h:P], ps[0:h], sinx_sb[h:P], mybir.AluOpType.mult
                    )
                    nc.vector.tensor_tensor(dst, t1, t2, mybir.AluOpType.add)

                def p1_tb(tb, qpool, hbmajor=False):
                    t0 = tb * QB
                    if tb + 1 < TB and tb + 1 not in xbuf:
                        xbuf[tb + 1] = load_x(tb + 1)
                    cos_sb, sinx_sb, xts = xbuf.pop(tb)

                    if hbmajor:
                        # hb-major so PE consumes x/w tiles as DMA delivers
                        # them (warmup); needs all NHC q accumulators live.
                        q_pss = [
                            qpool.tile([P, QB], F32, tag="qps", name=f"qps{dq}")
                            for dq in range(NHC)
                        ]
                        k_ps = psk_p.tile([P, QB], F32, tag="kps")
                        for hb in range(HB):
                            for dq in range(NHC):
                                nc.tensor.matmul(
                                    q_pss[dq],
                                    wq_sb[hb][:, dq * P : (dq + 1) * P],
                                    xts[hb],
                                    start=(hb == 0),
                                    stop=(hb == HB - 1),
                                )
                            nc.tensor.matmul(
                                k_ps,
                                wk_sb[hb],
                                xts[hb],
                                start=(hb == 0),
                                stop=(hb == HB - 1),
                            )
                        for dq in range(NHC):
                            rope(qT[dq][:, t0 : t0 + QB], q_pss[dq], cos_sb, sinx_sb)
                        rope(kT[:, t0 : t0 + QB], k_ps, cos_sb, sinx_sb)
                    else:
                        # q projections (per 128-wide d block) + rope
                        for dq in range(NHC):
                            q_ps = qpool.tile([P, QB], F32, tag="qps")
                            for hb in range(HB):
                                nc.tensor.matmul(
                                    q_ps,
                                    wq_sb[hb][:, dq * P : (dq + 1) * P],
                                    xts[hb],
                                    start=(hb == 0),
                                    stop=(hb == HB - 1),
                                )
                            rope(qT[dq][:, t0 : t0 + QB], q_ps, cos_sb, sinx_sb)
                        # k projection + rope
                        k_ps = psk_p.tile([P, QB], F32, tag="kps")
                        for hb in range(HB):
                            nc.tensor.matmul(
                                k_ps,
                                wk_sb[hb],
                                xts[hb],
                                start=(hb == 0),
                                stop=(hb == HB - 1),
                            )
                        rope(kT[:, t0 : t0 + QB], k_ps, cos_sb, sinx_sb)
                    # v projection (natural [t, d] layout)
                    for i in range(QB // P):
                        v_ps = psv_p.tile([P, P], F32, tag="vps")
                        for hb in range(HB):
                            nc.tensor.matmul(
                                v_ps,
                                xts[hb][:, i * P : (i + 1) * P],
                                wv_sb[hb],
                                start=(hb == 0),
                                stop=(hb == HB - 1),
                            )
                        nc.scalar.copy(vt[tb * (QB // P) + i], v_ps)

                # batch-0 token blocks first (deep psum pool for warmup),
                # then batch-1 blocks interleaved with batch-0 attention
                # chunks (big q-blocks first)
                with tc.tile_pool(name="psqw", bufs=NHC, space="PSUM") as psqw_p:
                    for tb in range(QBB):
                        p1_tb(tb, psqw_p, hbmajor=(tb == 0))
                with (
                    tc.tile_pool(name="psSa", bufs=2, space="PSUM") as pssa_p,
                    tc.tile_pool(name="psOa", bufs=1, space="PSUM") as psoa_p,
                    tc.tile_pool(name="psLa", bufs=1, space="PSUM") as psla_p,
                ):
                    b0_chunks = list(range(QBB - 1, -1, -1))
                    rest_tbs = list(range(QBB, TB))
                    for j in range(max(len(b0_chunks), len(rest_tbs))):
                        if j < len(b0_chunks):
                            p2_chunk(b0_chunks[j], pssa_p, psoa_p, psla_p, 1, ones_bf, mask_sb)
                        if j < len(rest_tbs):
                            p1_tb(rest_tbs[j], psq_p)

            # ---------------- phase 2b+3: batch-1 attention chunks (paired
            # heads) interleaved with o_proj for the already-gathered chunks.
            with (
                tc.tile_pool(name="wo", bufs=1) as wo_p,
                tc.tile_pool(name="astr", bufs=HB + 4) as as_p,
                tc.tile_pool(name="yout", bufs=4) as y_p,
                tc.tile_pool(name="psS", bufs=2, space="PSUM") as pss_p,
                tc.tile_pool(name="psO", bufs=1, space="PSUM") as pso_p,
                tc.tile_pool(name="psL", bufs=1, space="PSUM") as psl_p,
                tc.tile_pool(name="psY", bufs=2, space="PSUM") as psy_p,
            ):
                wo_sb = [wo_p.tile([P, DQ], BF16, tag=f"wo{i}", name=f"wo{i}") for i in range(HB)]
                for hb in range(HB):
                    nc.sync.dma_start(out=wo_sb[hb], in_=wo_c[hb * P : (hb + 1) * P, :])

                def p3_chunk(c):
                    t0 = c * QB
                    at = []
                    for ha in range(HB):
                        a = as_p.tile([P, QB], BF16, tag="astream")
                        nc.scalar.dma_start(
                            out=a, in_=attn_full[c][ha * P : (ha + 1) * P, :]
                        )
                        at.append(a)
                    for i in range(QB // P):
                        y_ps = psy_p.tile([P, DQ], F32, tag="yps")
                        for ha in range(HB):
                            nc.tensor.matmul(
                                y_ps,
                                at[ha][:, i * P : (i + 1) * P],
                                wo_sb[ha],
                                start=(ha == 0),
                                stop=(ha == HB - 1),
                            )
                        y_sb = y_p.tile([P, DQ], F32, tag="ysb")
                        if i % 2 == 0:
                            nc.scalar.copy(y_sb, y_ps)
                        else:
                            nc.vector.tensor_copy(y_sb, y_ps)
                        nc.scalar.dma_start(
                            out=y_c[t0 + i * P : t0 + (i + 1) * P, :], in_=y_sb
                        )

                b0_done = list(range(QBB - 1, -1, -1))
                later = [
                    b * QBB + qb
                    for b in range(1, B)
                    for qb in range(QBB - 1, -1, -1)
                ]
                p3_queue = b0_done + later
                for j, c in enumerate(later):
                    p2_chunk(c, pss_p, pso_p, psl_p, HG2, ones_bf, mask_sb)
                    if j < len(b0_done):
                        p3_chunk(b0_done[j])
                for c in p3_queue[len(later):]:
                    p3_chunk(c)
    nc.finalize()
    return nc


def _prep_inputs(hidden_states, wq, wk, wv, wo, position_ids, B, S, H, NH, NKV):
    """Host-side: bf16 casts, x transpose, rope tables, causal masks,
    per-core slices."""
    BT = B * S
    NHC = NH // NCORES
    DQ = NHC * P
    RB = QB // KB

    bf = ml_dtypes.bfloat16
    HB = H // P
    TBc = BT // QB
    x2 = np.asarray(hidden_states).reshape(BT, H).astype(bf)
    # blocked transpose: xT[hb, tb] is a contiguous [128, QB] tile
    xT = np.ascontiguousarray(
        x2.T.reshape(HB, P, TBc, QB).transpose(0, 2, 1, 3)
    )
    wq_b, wk_b, wv_b, wo_b = (np.asarray(w).astype(bf) for w in (wq, wk, wv, wo))

    # rope tables in transposed layout, blocked per token superblock
    half = P // 2
    inv_freq = 1.0 / (10000.0 ** (np.arange(half, dtype=np.float64) / half))
    pos = np.asarray(position_ids).astype(np.float64).reshape(BT)  # [b*S+s]
    ang = pos[None, :] * inv_freq[:, None]  # [64, BT]
    cos_t = np.concatenate([np.cos(ang), np.cos(ang)], 0).astype(np.float32)
    sinx_t = np.concatenate([-np.sin(ang), np.sin(ang)], 0).astype(np.float32)
    cos_t = np.ascontiguousarray(cos_t.reshape(P, TBc, QB).transpose(1, 0, 2))
    sinx_t = np.ascontiguousarray(sinx_t.reshape(P, TBc, QB).transpose(1, 0, 2))

    # diagonal-block causal masks: mask[o][k, q] = 1 if o*KB + k <= q
    kk = np.arange(KB)[None, :, None]
    qq = np.arange(QB)[None, None, :]
    oo = np.arange(RB)[:, None, None]
    masks_t = ((oo * KB + kk) <= qq).astype(bf)
    ones_t = np.ones((P, P), bf)

    in_maps = []
    for c in range(NCORES):
        in_maps.append(
            {
                "xT": xT,
                "wq_c": np.ascontiguousarray(wq_b[:, c * DQ : (c + 1) * DQ]),
                "wk_c": np.ascontiguousarray(wk_b[:, c * P : (c + 1) * P]),
                "wv_c": np.ascontiguousarray(wv_b[:, c * P : (c + 1) * P]),
                "wo_c": np.ascontiguousarray(wo_b[:, c * DQ : (c + 1) * DQ]),
                "cos_t": cos_t,
                "sinx_t": sinx_t,
                "masks_t": masks_t,
                "ones_t": ones_t,
            }
        )
    return in_maps


def run(hidden_states, wq, wk, wv, wo, position_ids, B, S, H, NH, NKV, trace=False):
    key = (B, S, H, NH, NKV)
    if key not in _CACHE:
        _CACHE[key] = build_program(B, S, H, NH, NKV)
    nc = _CACHE[key]
    in_maps = _prep_inputs(
        hidden_states, wq, wk, wv, wo, position_ids, B, S, H, NH, NKV
    )
    res = run_bass_kernel_spmd(nc, in_maps, core_ids=list(range(NCORES)), trace=trace)
    y = np.concatenate([res.results[c]["y_c"] for c in range(NCORES)], axis=1)
    out = y.reshape(B, S, NH * P).astype(np.float32)
    return (out, res) if trace else (out, None)


def kernel(hidden_states, wq, wk, wv, wo, position_ids):
    out, _ = run(
        hidden_states, wq, wk, wv, wo, position_ids, 2, 2048, 4096, 32, 8
    )
    return out


# revision 52
# speedup vs baseline: 1.0460x; 1.0040x over previous
"""Tensor-parallel LlamaAttention (GQA + RoPE + causal) for 8 trn2 NeuronCores.

Sharding: column-parallel q/k/v by head (NH/8 q-heads, NKV/8 kv-heads per
core), attention computed locally per head, per-512-token-chunk AllGather of
the (transposed) attention output overlapped with compute, then
column-parallel o_proj (each core computes a 512-wide output-column slice);
host concatenates slices.

Layout:
  xT[hb, tb] host-transposed into contiguous [128, 512] tiles (no PE
              transposes, max-efficiency DMA)
  qT/kT[d,t] from projection matmuls (lhsT=W block, rhs=xT block) + rope
  v[t, d]    natural layout (lhsT=xT block slice, rhs=Wv block)
  S^T[k, q]  = matmul(lhsT=kT slice, rhs=qT slice), diagonal tiles trimmed
               to the causally-live columns
  P^T        = exp(scale*S^T) on ACT; triangle mask on diag cols via DVE
  L          = ones-matmul PSUM accumulation; off-diagonal P tiles pair-
               summed on DVE (bf16) first so PE streams half the columns
  O^T[d, q] += matmul(lhsT=v tile, rhs=P^T)
  attnT      = O^T * reciprocal_approx_fast(L)

Schedule (the part that matters):
  tb0 runs hb-major from a deep PSUM pool so PE consumes tiles as the DMA
  warmup delivers them; batch-0 attention chunks (heads sequential, small
  PSUM footprint) interleave into batch-1's projection token-blocks so the
  serialized AllGathers start ~200us early; batch-1 chunks then interleave
  with o_proj of already-gathered chunks. A dummy warmup AllGather absorbs
  first-collective init. All collective outputs are Shared-addr-space DRAM.
"""

import math
import sys

import numpy as np

sys.path.insert(0, "/opt/trn_rl_repo")

import ml_dtypes  # noqa: E402

from concourse import bacc, mybir, tile  # noqa: E402
from concourse.bass_utils import run_bass_kernel_spmd  # noqa: E402

F32 = mybir.dt.float32
BF16 = mybir.dt.bfloat16
NCORES = 8
P = 128  # partitions / head dim
QB = 512  # q-block (PSUM free dim)
KB = 128  # k-block (contraction tile)
_CACHE = {}


def build_program(B, S, H, NH, NKV):
    """Build the per-core Bass program. All cores run the same program on
    different weight slices."""
    nc = bacc.Bacc("TRN2", num_devices=NCORES)

    BT = B * S  # total tokens
    NHC = NH // NCORES  # q heads per core
    NKC = NKV // NCORES  # kv heads per core
    assert NKC == 1
    DQ = NHC * P  # per-core q width
    HB = H // P  # h blocks
    TB = BT // QB  # token super-blocks
    QBB = S // QB  # q blocks per batch
    RB = QB // KB  # diag tiles per q block

    xT = nc.declare_dram_parameter("xT", [HB, BT // QB, P, QB], BF16, isOutput=False)
    wq_c = nc.declare_dram_parameter("wq_c", [H, DQ], BF16, isOutput=False)
    wk_c = nc.declare_dram_parameter("wk_c", [H, P], BF16, isOutput=False)
    wv_c = nc.declare_dram_parameter("wv_c", [H, P], BF16, isOutput=False)
    wo_c = nc.declare_dram_parameter("wo_c", [H, DQ], BF16, isOutput=False)
    cos_t = nc.declare_dram_parameter("cos_t", [BT // QB, P, QB], F32, isOutput=False)
    sinx_t = nc.declare_dram_parameter("sinx_t", [BT // QB, P, QB], F32, isOutput=False)
    masks_t = nc.declare_dram_parameter("masks_t", [RB, P, QB], BF16, isOutput=False)
    ones_t = nc.declare_dram_parameter("ones_t", [P, P], BF16, isOutput=False)
    y_c = nc.declare_dram_parameter("y_c", [BT, DQ], F32, isOutput=True)

    scale = 1.0 / math.sqrt(P)
    nchunks = B * QBB  # attention/o_proj pipeline chunks (QB tokens each)

    with tile.TileContext(nc) as tc:
        with (
            tc.tile_pool(name="dram", bufs=1, space="DRAM") as dram,
            tc.tile_pool(name="const", bufs=1) as constp,
            tc.tile_pool(name="persist", bufs=1) as persist,
            tc.tile_pool(name="pP", bufs=6) as p_p,
            tc.tile_pool(name="pPair", bufs=4) as pp_p,
            tc.tile_pool(name="rinvp", bufs=2) as r_p,
            tc.tile_pool(name="aout", bufs=4) as ao_p,
        ):
            # dummy collective during phase 1 absorbs first-collective init
            cc_warm_in = dram.tile([P, 8], BF16, tag="ccwi", name="ccwi")
            cc_warm_out = dram.tile(
                [NCORES * P, 8], BF16, tag="ccwo", name="ccwo", addr_space="Shared"
            )
            attn_loc = [
                dram.tile([DQ, QB], BF16, tag=f"attn_loc{c}", name=f"attn_loc{c}")
                for c in range(nchunks)
            ]
            attn_full = [
                dram.tile(
                    [NCORES * DQ, QB],
                    BF16,
                    tag=f"attn_full{c}",
                    name=f"attn_full{c}",
                    addr_space="Shared",
                )
                for c in range(nchunks)
            ]

            # persistent per-core activations (bf16)
            qT = [persist.tile([P, BT], BF16, tag=f"qT{i}", name=f"qT{i}") for i in range(NHC)]
            kT = persist.tile([P, BT], BF16, tag="kT")
            vt = [persist.tile([P, P], BF16, tag=f"v{i}", name=f"v{i}") for i in range(BT // P)]

            HG2 = min(2, NHC)

            def p2_chunk(c, pss_p, pso_p, psl_p, HG, ones_bf, mask_sb):
                b, qb = divmod(c, QBB)
                q0 = b * S + qb * QB
                nkb = (qb + 1) * RB
                for hp in range(NHC // HG):
                    heads = range(hp * HG, (hp + 1) * HG)
                    o_ps = {
                        h: pso_p.tile([P, QB], F32, tag=f"ops{h % HG}", name=f"ops{h % HG}")
                        for h in heads
                    }
                    l_ps = {
                        h: psl_p.tile([P, QB], F32, tag=f"lps{h % HG}", name=f"lps{h % HG}")
                        for h in heads
                    }
                    # L bookkeeping: off-diagonal P tiles are summed in pairs
                    # on DVE (bf16) so PE streams half the columns through the
                    # ones-matmul; diagonal tiles go straight to PE (trimmed).
                    # Emission of each L-matmul is deferred ~one k-iteration so
                    # the in-order PE queue never waits on the DVE producer.
                    lfirst = {h: True for h in heads}
                    pend = {h: None for h in heads}
                    lq = []  # deferred (h, rhs, w0) L-matmul jobs

                    def l_mm_raw(h, rhs, w0, stop):
                        nc.tensor.matmul(
                            l_ps[h][:, w0:QB],
                            ones_bf,
                            rhs[:, w0:QB],
                            start=lfirst[h],
                            stop=stop,
                        )
                        lfirst[h] = False

                    def l_mm(h, rhs, w0, stop=False):
                        lq.append((h, rhs, w0))

                    def l_flush(keep):
                        while len(lq) > keep:
                            h, rhs, w0 = lq.pop(0)
                            l_mm_raw(h, rhs, w0, False)

                    for kb in range(nkb):
                        l_flush(len(heads))
                        k0 = b * S + kb * KB
                        o = kb - qb * RB  # >=0 -> diagonal tile
                        # causal: first o*KB columns of this q-block see
                        # nothing from this k-tile; compute only the rest.
                        w0 = o * KB if o > 0 else 0
                        p_sbs = {}
                        for h in heads:
                            s_ps = pss_p.tile([P, QB], F32, tag="sps")
                            nc.tensor.matmul(
                                s_ps[:, w0:QB],
                                kT[:, k0 : k0 + KB],
                                qT[h][:, q0 + w0 : q0 + QB],
                                start=True,
                                stop=True,
                            )
                            p_sb = p_p.tile([P, QB], BF16, tag="P")
                            nc.scalar.activation(
                                p_sb[:, w0:QB],
                                s_ps[:, w0:QB],
                                mybir.ActivationFunctionType.Exp,
                                scale=scale,
                            )
                            if o >= 0:
                                # triangle mask on the 128 diagonal cols
                                nc.vector.tensor_tensor(
                                    p_sb[:, w0 : w0 + KB],
                                    p_sb[:, w0 : w0 + KB],
                                    mask_sb,
                                    mybir.AluOpType.mult,
                                )
                            p_sbs[h] = p_sb
                        for h in heads:
                            nc.tensor.matmul(
                                o_ps[h][:, w0:QB],
                                vt[(b * S + kb * KB) // P],
                                p_sbs[h][:, w0:QB],
                                start=(kb == 0),
                                stop=(kb == nkb - 1),
                            )
                            if o >= 0:
                                if pend[h] is not None:  # odd off-diag count
                                    l_mm(h, pend[h], 0, False)
                                    pend[h] = None
                                l_mm(h, p_sbs[h], w0, kb == nkb - 1)
                            elif pend[h] is None:
                                pend[h] = p_sbs[h]
                            else:
                                ppair = pp_p.tile([P, QB], BF16, tag="ppair")
                                nc.vector.tensor_tensor(
                                    ppair, pend[h], p_sbs[h], mybir.AluOpType.add
                                )
                                pend[h] = None
                                l_mm(h, ppair, 0, False)
                    last_idx = {}
                    for i, (h, _, _) in enumerate(lq):
                        last_idx[h] = i
                    for i, (h, rhs, w0) in enumerate(lq):
                        l_mm_raw(h, rhs, w0, i == last_idx[h])
                    lq.clear()
                    for h in heads:
                        rinv = r_p.tile([P, QB], F32, tag="rinv")
                        nc.vector.reciprocal_approx_fast(rinv, l_ps[h])
                        attn_sb = ao_p.tile([P, QB], BF16, tag="attn")
                        nc.vector.tensor_tensor(
                            attn_sb, o_ps[h], rinv, mybir.AluOpType.mult
                        )
                        nc.sync.dma_start(
                            out=attn_loc[c][h * P : (h + 1) * P, :], in_=attn_sb
                        )
                nc.gpsimd.collective_compute(
                    "AllGather",
                    mybir.AluOpType.bypass,
                    replica_groups=[list(range(NCORES))],
                    ins=[attn_loc[c][:, :]],
                    outs=[attn_full[c][:, :]],
                )

            # ---------------- phase 1: q/k/v projections + rope, with batch-0
            # attention chunks (and their collectives) interleaved into the
            # second half so the AllGathers start as early as possible.
            with (
                tc.tile_pool(name="xs", bufs=2 * HB + 4) as xs_p,
                tc.tile_pool(name="wqkv", bufs=1) as w_p,
                tc.tile_pool(name="tabs", bufs=2) as tab_p,
                tc.tile_pool(name="ropetmp", bufs=4) as rt_p,
                tc.tile_pool(name="psq", bufs=2, space="PSUM") as psq_p,
                tc.tile_pool(name="psk", bufs=1, space="PSUM") as psk_p,
                tc.tile_pool(name="psv", bufs=1, space="PSUM") as psv_p,
            ):
                # DMA issue order matters for warmup: first token-block's x
                # tiles and wq interleaved so PE can start right away, then
                # the second token-block's x, then the remaining weights.
                cos_sb0 = tab_p.tile([P, QB], F32, tag="cos", name="cos0")
                sinx_sb0 = tab_p.tile([P, QB], F32, tag="sinx", name="sinx0")
                nc.sync.dma_start(out=cos_sb0, in_=cos_t[0])
                nc.sync.dma_start(out=sinx_sb0, in_=sinx_t[0])
                wq_sb = [w_p.tile([P, DQ], BF16, tag=f"wq{i}", name=f"wq{i}") for i in range(HB)]
                wk_sb = [w_p.tile([P, P], BF16, tag=f"wk{i}", name=f"wk{i}") for i in range(HB)]
                wv_sb = [w_p.tile([P, P], BF16, tag=f"wv{i}", name=f"wv{i}") for i in range(HB)]

                def load_x(tb):
                    cos_sb = tab_p.tile([P, QB], F32, tag="cos", name=f"cos{tb}")
                    sinx_sb = tab_p.tile([P, QB], F32, tag="sinx", name=f"sinx{tb}")
                    nc.sync.dma_start(out=cos_sb, in_=cos_t[tb])
                    nc.sync.dma_start(out=sinx_sb, in_=sinx_t[tb])
                    xts = []
                    for hb in range(HB):
                        xi = xs_p.tile([P, QB], BF16, tag="xs", name=f"x{tb}_{hb}")
                        nc.sync.dma_start(out=xi, in_=xT[hb, tb])
                        xts.append(xi)
                    return cos_sb, sinx_sb, xts

                xbuf = {}
                xts0 = []
                for hb in range(HB):
                    xi = xs_p.tile([P, QB], BF16, tag="xs", name=f"x0_{hb}")
                    nc.sync.dma_start(out=xi, in_=xT[hb, 0])
                    xts0.append(xi)
                    nc.sync.dma_start(out=wq_sb[hb], in_=wq_c[hb * P : (hb + 1) * P, :])
                    nc.sync.dma_start(out=wk_sb[hb], in_=wk_c[hb * P : (hb + 1) * P, :])
                xbuf[0] = (cos_sb0, sinx_sb0, xts0)
                if TB > 1:
                    xbuf[1] = load_x(1)
                for hb in range(HB):
                    nc.sync.dma_start(out=wv_sb[hb], in_=wv_c[hb * P : (hb + 1) * P, :])
                # constants for phase 2 (small; needed later)
                ones_bf = constp.tile([P, P], BF16, tag="ones")
                nc.sync.dma_start(out=ones_bf, in_=ones_t[:, :])
                mask_sb = constp.tile([P, P], BF16, tag="mask")
                nc.sync.dma_start(out=mask_sb, in_=masks_t[0, :, 0:P])
                nc.sync.dma_start(out=cc_warm_in[:, :], in_=ones_t[:, 0:8])
                nc.gpsimd.collective_compute(
                    "AllGather",
                    mybir.AluOpType.bypass,
                    replica_groups=[list(range(NCORES))],
                    ins=[cc_warm_in[:, :]],
                    outs=[cc_warm_out[:, :]],
                )

                def rope(dst, ps, cos_sb, sinx_sb):
                    """dst[:, t0:t0+QB] = ps*cos + shift64(ps)*sinx (all [128,QB])"""
                    t1 = rt_p.tile([P, QB], F32, tag="ropet1")
                    t2 = rt_p.tile([P, QB], F32, tag="ropet2")
                    nc.vector.tensor_tensor(t1, ps, cos_sb, mybir.AluOpType.mult)
                    h = P // 2
                    nc.vector.tensor_tensor(
                        t2[0:h], ps[h:P], sinx_sb[0:h], mybir.AluOpType.mult
                    )
                    nc.vector.tensor_tensor(
                        t2[h:P], ps[0:h], sinx_sb[h:P], mybir.AluOpType.mult
                    )
                    nc.vector.tensor_tensor(dst, t1, t2, mybir.AluOpType.add)

                def p1_tb(tb, qpool, hbmajor=False):
                    t0 = tb * QB
                    if tb + 1 < TB and tb + 1 not in xbuf:
                        xbuf[tb + 1] = load_x(tb + 1)
                    cos_sb, sinx_sb, xts = xbuf.pop(tb)

                    if hbmajor:
                        # hb-major so PE consumes x/w tiles as DMA delivers
                        # them (warmup); needs all NHC q accumulators live.
                        q_pss = [
                            qpool.tile([P, QB], F32, tag="qps", name=f"qps{dq}")
                            for dq in range(NHC)
                        ]
                        k_ps = psk_p.tile([P, QB], F32, tag="kps")
                        for hb in range(HB):
                            for dq in range(NHC):
                                nc.tensor.matmul(
                                    q_pss[dq],
                                    wq_sb[hb][:, dq * P : (dq + 1) * P],
                                    xts[hb],
                                    start=(hb == 0),
                                    stop=(hb == HB - 1),
                                )
                            nc.tensor.matmul(
                                k_ps,
                                wk_sb[hb],
                                xts[hb],
                                start=(hb == 0),
                                stop=(hb == HB - 1),
                            )
                        for dq in range(NHC):
                            rope(qT[dq][:, t0 : t0 + QB], q_pss[dq], cos_sb, sinx_sb)
                        rope(kT[:, t0 : t0 + QB], k_ps, cos_sb, sinx_sb)
                    else:
                        # q projections (per 128-wide d block) + rope
                        for dq in range(NHC):
                            q_ps = qpool.tile([P, QB], F32, tag="qps")
                            for hb in range(HB):
                                nc.tensor.matmul(
                                    q_ps,
                                    wq_sb[hb][:, dq * P : (dq + 1) * P],
                                    xts[hb],
                                    start=(hb == 0),
                                    stop=(hb == HB - 1),
                                )
                            rope(qT[dq][:, t0 : t0 + QB], q_ps, cos_sb, sinx_sb)
                        # k projection + rope
                        k_ps = psk_p.tile([P, QB], F32, tag="kps")
                        for hb in range(HB):
                            nc.tensor.matmul(
                                k_ps,
                                wk_sb[hb],
                                xts[hb],
                                start=(hb == 0),
                                stop=(hb == HB - 1),
                            )
                        rope(kT[:, t0 : t0 + QB], k_ps, cos_sb, sinx_sb)
                    # v projection (natural [t, d] layout)
                    for i in range(QB // P):
                        v_ps = psv_p.tile([P, P], F32, tag="vps")
                        for hb in range(HB):
                            nc.tensor.matmul(
                                v_ps,
                                xts[hb][:, i * P : (i + 1) * P],
                                wv_sb[hb],
                                start=(hb == 0),
                                stop=(hb == HB - 1),
                            )
                        nc.scalar.copy(vt[tb * (QB // P) + i], v_ps)

                # batch-0 token blocks first (deep psum pool for warmup),
                # then batch-1 blocks interleaved with batch-0 attention
                # chunks (big q-blocks first)
                with tc.tile_pool(name="psqw", bufs=NHC, space="PSUM") as psqw_p:
                    for tb in range(QBB):
                        p1_tb(tb, psqw_p, hbmajor=(tb == 0))
                with (
                    tc.tile_pool(name="psSa", bufs=2, space="PSUM") as pssa_p,
                    tc.tile_pool(name="psOa", bufs=1, space="PSUM") as psoa_p,
                    tc.tile_pool(name="psLa", bufs=1, space="PSUM") as psla_p,
                ):
                    b0_chunks = list(range(QBB - 1, -1, -1))
                    rest_tbs = list(range(QBB, TB))
                    for j in range(max(len(b0_chunks), len(rest_tbs))):
                        if j < len(b0_chunks):
                            p2_chunk(b0_chunks[j], pssa_p, psoa_p, psla_p, 1, ones_bf, mask_sb)
                        if j < len(rest_tbs):
                            p1_tb(rest_tbs[j], psq_p)

            # ---------------- phase 2b+3: batch-1 attention chunks (paired
            # heads) interleaved with o_proj for the already-gathered chunks.
            with (
                tc.tile_pool(name="wo", bufs=1) as wo_p,
                tc.tile_pool(name="astr", bufs=HB + 4) as as_p,
                tc.tile_pool(name="yout", bufs=4) as y_p,
                tc.tile_pool(name="psS", bufs=2, space="PSUM") as pss_p,
                tc.tile_pool(name="psO", bufs=1, space="PSUM") as pso_p,
                tc.tile_pool(name="psL", bufs=1, space="PSUM") as psl_p,
                tc.tile_pool(name="psY", bufs=2, space="PSUM") as psy_p,
            ):
                wo_sb = [wo_p.tile([P, DQ], BF16, tag=f"wo{i}", name=f"wo{i}") for i in range(HB)]
                for hb in range(HB):
                    nc.sync.dma_start(out=wo_sb[hb], in_=wo_c[hb * P : (hb + 1) * P, :])

                def p3_chunk(c):
                    t0 = c * QB
                    at = []
                    for ha in range(HB):
                        a = as_p.tile([P, QB], BF16, tag="astream")
                        nc.scalar.dma_start(
                            out=a, in_=attn_full[c][ha * P : (ha + 1) * P, :]
                        )
                        at.append(a)
                    for i in range(QB // P):
                        y_ps = psy_p.tile([P, DQ], F32, tag="yps")
                        for ha in range(HB):
                            nc.tensor.matmul(
                                y_ps,
                                at[ha][:, i * P : (i + 1) * P],
                                wo_sb[ha],
                                start=(ha == 0),
                                stop=(ha == HB - 1),
                            )
                        y_sb = y_p.tile([P, DQ], F32, tag="ysb")
                        if i % 2 == 0:
                            nc.scalar.copy(y_sb, y_ps)
                        else:
                            nc.vector.tensor_copy(y_sb, y_ps)
                        nc.scalar.dma_start(
                            out=y_c[t0 + i * P : t0 + (i + 1) * P, :], in_=y_sb
                        )

                b0_done = list(range(QBB - 1, -1, -1))
                later = [
                    b * QBB + qb
                    for b in range(1, B)
                    for qb in range(QBB - 1, -1, -1)
                ]
                p3_queue = b0_done + later
                for j, c in enumerate(later):
                    p2_chunk(c, pss_p, pso_p, psl_p, HG2, ones_bf, mask_sb)
                    if j < len(b0_done):
                        p3_chunk(b0_done[j])
                for c in p3_queue[len(later):]:
                    p3_chunk(c)
    nc.finalize()
    return nc


def _prep_inputs(hidden_states, wq, wk, wv, wo, position_ids, B, S, H, NH, NKV):
    """Host-side: bf16 casts, x transpose, rope tables, causal masks,
    per-core slices."""
    BT = B * S
    NHC = NH // NCORES
    DQ = NHC * P
    RB = QB // KB

    bf = ml_dtypes.bfloat16
    HB = H // P
    TBc = BT // QB
    x2 = np.asarray(hidden_states).reshape(BT, H).astype(bf)
    # blocked transpose: xT[hb, tb] is a contiguous [128, QB] tile
    xT = np.ascontiguousarray(
        x2.T.reshape(HB, P, TBc, QB).transpose(0, 2, 1, 3)
    )
    wq_b, wk_b, wv_b, wo_b = (np.asarray(w).astype(bf) for w in (wq, wk, wv, wo))

    # rope tables in transposed layout, blocked per token superblock
    half = P // 2
    inv_freq = 1.0 / (10000.0 ** (np.arange(half, dtype=np.float64) / half))
    pos = np.asarray(position_ids).astype(np.float64).reshape(BT)  # [b*S+s]
    ang = pos[None, :] * inv_freq[:, None]  # [64, BT]
    cos_t = np.concatenate([np.cos(ang), np.cos(ang)], 0).astype(np.float32)
    sinx_t = np.concatenate([-np.sin(ang), np.sin(ang)], 0).astype(np.float32)
    cos_t = np.ascontiguousarray(cos_t.reshape(P, TBc, QB).transpose(1, 0, 2))
    sinx_t = np.ascontiguousarray(sinx_t.reshape(P, TBc, QB).transpose(1, 0, 2))

    # diagonal-block causal masks: mask[o][k, q] = 1 if o*KB + k <= q
    kk = np.arange(KB)[None, :, None]
    qq = np.arange(QB)[None, None, :]
    oo = np.arange(RB)[:, None, None]
    masks_t = ((oo * KB + kk) <= qq).astype(bf)
    ones_t = np.ones((P, P), bf)

    in_maps = []
    for c in range(NCORES):
        in_maps.append(
            {
                "xT": xT,
                "wq_c": np.ascontiguousarray(wq_b[:, c * DQ : (c + 1) * DQ]),
                "wk_c": np.ascontiguousarray(wk_b[:, c * P : (c + 1) * P]),
                "wv_c": np.ascontiguousarray(wv_b[:, c * P : (c + 1) * P]),
                "wo_c": np.ascontiguousarray(wo_b[:, c * DQ : (c + 1) * DQ]),
                "cos_t": cos_t,
                "sinx_t": sinx_t,
                "masks_t": masks_t,
                "ones_t": ones_t,
            }
        )
    return in_maps


def run(hidden_states, wq, wk, wv, wo, position_ids, B, S, H, NH, NKV, trace=False):
    key = (B, S, H, NH, NKV)
    if key not in _CACHE:
        _CACHE[key] = build_program(B, S, H, NH, NKV)
    nc = _CACHE[key]
    in_maps = _prep_inputs(
        hidden_states, wq, wk, wv, wo, position_ids, B, S, H, NH, NKV
    )
    res = run_bass_kernel_spmd(nc, in_maps, core_ids=list(range(NCORES)), trace=trace)
    y = np.concatenate([res.results[c]["y_c"] for c in range(NCORES)], axis=1)
    out = y.reshape(B, S, NH * P).astype(np.float32)
    return (out, res) if trace else (out, None)


def kernel(hidden_states, wq, wk, wv, wo, position_ids):
    out, _ = run(
        hidden_states, wq, wk, wv, wo, position_ids, 2, 2048, 4096, 32, 8
    )
    return out
